# revision 1
# baseline (speedup 1.0000x reference)
"""Distributed attention kernel for Trainium2 (8 NeuronCores).

Reference computation (B=2, N=2048, C=1024, H=16, D=64, ALPHA=0.5):
    qkv = x @ W_qkv -> q,k,v [B,H,N,D]
    attn = softmax(q @ k^T / sqrt(D))
    attn = 0.5*dm + 0.5*attn
    out  = (attn @ v).reshape(B,N,C) @ W_proj + b_proj

Sharding: 8 cores = 2 batches x 4 head-groups (4 heads each).
Each core computes its head-group's slice end-to-end, including a partial
projection (row-slice of W_proj); host sums the 4 partials per batch.

On-device layout strategy (per core):
  - x arrives transposed [C, N] so the C-contraction has C on partitions.
  - q,k are produced transposed [Dg=256, N] (head-dim on partitions).
  - scores are computed transposed: S^T[k',q] = k^T.T @ q^T, so softmax's
    exp runs on ScalarE straight out of PSUM and the sum-over-k' is folded
    into the attn@v matmul via a ones-column appended to v (lhsT=[m, 65]:
    row 64 of the PSUM accumulator receives sum_m e[m,q] = the softmax
    denominator) -- no extra reduction pass over the N^2 matrix.
  - max-subtraction is skipped: scores are ~N(0,1), |s| < ~8 << 88, so
    exp never overflows in fp32.
  - dm is pre-halved + transposed on host and accumulated through its own
    matmul stream with v as the stationary operand.
  - the ones column holds 2.0, so the accumulator row is 2r and the
    normalization constant 0.5/r is a plain reciprocal.
  - normalization (per q column) is applied after attn@v on the small
    [64, 512] output tiles; the row vector 0.5/r is broadcast across
    partitions via a DRAM-bounce DMA (0-step partition APs are only legal
    on the DRAM side), or via a tiny fp16 PE matmul on the final chunk
    where the PE is idle.
  - all matmul operands are fp16 (1 cycle/row at the PE like bf16, but
    10-bit mantissa); PSUM accumulation stays fp32.
  - scores for a head pair land in one [128,1024] PSUM tile so each exp
    covers two heads (halves ScalarE instruction overhead -- ScalarE exp
    over the N^2 scores is the second-busiest engine after the PE).
"""

import numpy as np

B, N, C, H, D = 2, 2048, 1024, 16, 64
NCORES = 8
HG = 4                # head-groups per batch
HPC = H // HG         # heads per core = 4
DG = HPC * D          # 256: head-group width
SCALE = D ** -0.5

KT = C // 128         # 8 contraction tiles for qkv/x
NQ = N // 512         # 4 q-chunks
MT = N // 128         # 16 m (key) tiles


def _build_program():
    import concourse.bass as bass
    import concourse.bacc as bacc
    import concourse.tile as tile
    from concourse import mybir
    from contextlib import ExitStack

    f32 = mybir.dt.float32
    Exp = mybir.ActivationFunctionType.Exp
    f16 = mybir.dt.float16

    nc = bacc.Bacc()
    xT = nc.declare_dram_parameter("xT", [C, N], f16, isOutput=False)
    wq = nc.declare_dram_parameter("wq", [C, DG], f16, isOutput=False)
    wk = nc.declare_dram_parameter("wk", [C, DG], f16, isOutput=False)
    wv = nc.declare_dram_parameter("wv", [C, DG], f16, isOutput=False)
    wp = nc.declare_dram_parameter("wp", [DG, C], f16, isOutput=False)
    dmt = nc.declare_dram_parameter("dmt", [N, N], f16, isOutput=False)
    pout = nc.declare_dram_parameter("pout", [C, N], f16, isOutput=True)

    with tile.TileContext(nc) as tc, ExitStack() as ctx:
        big = ctx.enter_context(tc.tile_pool(name="big", bufs=1))
        epool = ctx.enter_context(tc.tile_pool(name="epool", bufs=6))
        small = ctx.enter_context(tc.tile_pool(name="small", bufs=2))
        outp = ctx.enter_context(tc.tile_pool(name="outp", bufs=4))
        # PSUM: psS slot [128,1024] x2 (4 banks) + pe0/pe1 (2) + pd0 (1) = 7 banks
        psS = ctx.enter_context(tc.tile_pool(name="psS", bufs=2, space="PSUM"))
        psE = ctx.enter_context(tc.tile_pool(name="psE", bufs=1, space="PSUM"))
        psD = ctx.enter_context(tc.tile_pool(name="psD", bufs=2, space="PSUM"))

        xt = big.tile([128, KT, N], f16)
        wq_s = big.tile([128, KT, DG], f16)
        wk_s = big.tile([128, KT, DG], f16)
        wv_s = big.tile([128, KT, DG], f16)
        qt = big.tile([128, 2, N], f16)
        kt = big.tile([128, 2, N], f16)
        vaug = big.tile([128, MT, HPC, D + 1], f16)
        vb = big.tile([128, MT, DG], f16)
        outT = big.tile([128, 2, N], f16)
        wp_s = big.tile([128, 2, C], f16)
        ones_sb = big.tile([128, MT * HPC], f32)
        ones16 = big.tile([1, D], f16)
        dms = big.tile([128, MT, N], f16)
        rscratch = nc.dram_tensor("rscratch", [8, 1024], f32)

        nc.vector.memset(ones_sb[:, :], 2.0)
        nc.vector.memset(ones16[:, :], 1.0)
        nc.vector.tensor_copy(vaug[:, :, :, D], ones_sb[:, :])

        for ct in range(KT):
            if ct == 0:
                nc.sync.dma_start(out=xt[:, 0, 0:1024], in_=xT[0:128, 0:1024])
                nc.sync.dma_start(out=xt[:, 0, 1024:2048], in_=xT[0:128, 1024:2048])
            else:
                nc.sync.dma_start(out=xt[:, ct, :], in_=xT[ct * 128:(ct + 1) * 128, :])
            nc.sync.dma_start(out=wk_s[:, ct, :], in_=wk[ct * 128:(ct + 1) * 128, :])
        for ct in range(KT):
            nc.sync.dma_start(out=wv_s[:, ct, :], in_=wv[ct * 128:(ct + 1) * 128, :])
        for ct in range(KT):
            nc.sync.dma_start(out=wq_s[:, ct, :], in_=wq[ct * 128:(ct + 1) * 128, :])
        for jo in range(2):
            nc.sync.dma_start(out=wp_s[:, jo, :], in_=wp[jo * 128:(jo + 1) * 128, :])
        for mt in range(MT):
            nc.sync.dma_start(out=dms[:, mt, :], in_=dmt[mt * 128:(mt + 1) * 128, :])

        # ---- phase 1: k^T first, then v, then q^T (attn consumers need k/v whole) ----
        def qk_proj(w_s, dst, scale, goff):
            for jo in range(2):
                for nq in range(NQ):
                    g = goff + jo * NQ + nq
                    ps = psS.tile([128, 512], f32, name="ps", tag="psS")
                    for i in range(KT):
                        ct = (g + i) % KT
                        nc.tensor.matmul(
                            ps[:, :],
                            lhsT=w_s[:, ct, jo * 128:(jo + 1) * 128],
                            rhs=xt[:, ct, nq * 512:(nq + 1) * 512],
                            start=(i == 0), stop=(i == KT - 1),
                        )
                    if scale != 1.0:
                        nc.vector.tensor_scalar_mul(
                            dst[:, jo, nq * 512:(nq + 1) * 512], ps[:, :], scale)
                    else:
                        nc.vector.tensor_copy(dst[:, jo, nq * 512:(nq + 1) * 512], ps[:, :])

        # k^T: first 6 output groups accumulate ct-outer across 6 PSUM slots so
        # each arriving xt tile feeds 6 matmuls (PE keeps pace with the DMA).
        kgroups = [(jo, nq) for jo in range(2) for nq in range(NQ)]
        ktags = ["psS", "psS", "pe0", "pe1", "pd0", "pd0"]
        kps = {}
        for i, g in enumerate(kgroups[:6]):
            if ktags[i] in ("pe0", "pe1"):
                kps[g] = psE.tile([128, 512], f32, name=f"kp{i}", tag=ktags[i])
            elif ktags[i] == "pd0":
                kps[g] = psD.tile([128, 512], f32, name=f"kp{i}", tag="pd0")
            else:
                kps[g] = psS.tile([128, 512], f32, name=f"kp{i}", tag="psS")
        for ct in range(KT):
            for jo, nq in kgroups[:6]:
                nc.tensor.matmul(
                    kps[(jo, nq)][:, :],
                    lhsT=wk_s[:, ct, jo * 128:(jo + 1) * 128],
                    rhs=xt[:, ct, nq * 512:(nq + 1) * 512],
                    start=(ct == 0), stop=(ct == KT - 1),
                )
        corder = sorted(range(6), key=lambda i: 0 if ktags[i] in ("pe0", "pe1") else 1)
        for i in corder:
            jo, nq = kgroups[i]
            nc.vector.tensor_copy(kt[:, jo, nq * 512:(nq + 1) * 512], kps[(jo, nq)][:, :])
        for jo, nq in kgroups[6:]:
            ps = psS.tile([128, 512], f32, name="ps", tag="psS")
            for i in range(KT):
                ct = (nq + i) % KT
                nc.tensor.matmul(
                    ps[:, :],
                    lhsT=wk_s[:, ct, jo * 128:(jo + 1) * 128],
                    rhs=xt[:, ct, nq * 512:(nq + 1) * 512],
                    start=(i == 0), stop=(i == KT - 1),
                )
            nc.vector.tensor_copy(kt[:, jo, nq * 512:(nq + 1) * 512], ps[:, :])

        for mt in range(MT):
            ps = psE.tile([128, DG], f32, name="ps", tag=f"pe{mt % 2}", padded_shape=[128, 512])
            for i in range(KT):
                ct = (mt + i) % KT
                nc.tensor.matmul(
                    ps[:, :],
                    lhsT=xt[:, ct, mt * 128:(mt + 1) * 128],
                    rhs=wv_s[:, ct, :],
                    start=(i == 0), stop=(i == KT - 1),
                )
            nc.vector.tensor_copy(vaug[:, mt, :, 0:D], ps[:, :])
            nc.vector.tensor_copy(vb[:, mt, :], ps[:, :])

        qk_proj(wq_s, qt, SCALE, 4)

        # ---- phase 2: attention, 2 heads (one k/q partition tile) per pass ----
        def proj_group(nq, co):
            qsl = slice(nq * 512, (nq + 1) * 512)
            ps = psD.tile([128, 512], f32, name="ps", tag="pd0")
            for jo in range(2):
                nc.tensor.matmul(
                    ps[:, :],
                    lhsT=wp_s[:, jo, co * 128:(co + 1) * 128],
                    rhs=outT[:, jo, qsl],
                    start=(jo == 0), stop=(jo == 1),
                )
            so = outp.tile([128, 512], f16)
            nc.vector.tensor_copy(so[:, :], ps[:, :])
            nc.sync.dma_start(out=pout[co * 128:(co + 1) * 128, qsl], in_=so[:, :])

        pending_proj = None
        for nq in range(NQ):
            qsl = slice(nq * 512, (nq + 1) * 512)
            for hp in range(2):
                pe0 = psE.tile([D + 1, 512], f32, name="pe0", tag="pe0")
                pe1 = psE.tile([D + 1, 512], f32, name="pe1", tag="pe1")
                pd = psD.tile([128, 512], f32, name="pd", tag="pd0")
                for mt in range(MT):
                    msl = slice(mt * 128, (mt + 1) * 128)
                    nc.tensor.matmul(
                        pd[:, :],
                        lhsT=vb[:, mt, hp * 128:(hp + 1) * 128],
                        rhs=dms[:, mt, qsl],
                        start=(mt == 0), stop=(mt == MT - 1),
                    )
                    sps = psS.tile([128, 1024], f32, name="sps", tag="psS")
                    nc.tensor.matmul(
                        sps[:, 0:512],
                        lhsT=kt[0:D, hp, msl], rhs=qt[0:D, hp, qsl],
                        start=True, stop=True,
                    )
                    nc.tensor.matmul(
                        sps[:, 512:1024],
                        lhsT=kt[D:2 * D, hp, msl], rhs=qt[D:2 * D, hp, qsl],
                        start=True, stop=True,
                    )
                    et = epool.tile([128, 1024], f16)
                    nc.scalar.activation(et[:, :], sps[:, :], Exp)
                    nc.tensor.matmul(
                        pe0[:, :], lhsT=vaug[:, mt, 2 * hp, :], rhs=et[:, 0:512],
                        start=(mt == 0), stop=(mt == MT - 1),
                    )
                    nc.tensor.matmul(
                        pe1[:, :], lhsT=vaug[:, mt, 2 * hp + 1, :], rhs=et[:, 512:1024],
                        start=(mt == 0), stop=(mt == MT - 1),
                    )
                    if pending_proj is not None and hp == 0 and 1 <= mt <= 8:
                        proj_group(pending_proj, mt - 1)
                # epilogue. Non-last chunks: free the PSUM banks with quick
                # copies, then normalize off the critical path (0.5/r broadcast
                # via DRAM bounce). Last chunk: nothing needs the banks again,
                # so read the accumulators directly and broadcast via a tiny
                # fp16 PE matmul (the PE is idle in the tail).
                slot = nq * 2 + hp
                last = (nq == NQ - 1 and hp == 1)
                if last:
                    pe_s0, pe_s1, pd_s = pe0, pe1, pd
                else:
                    pe_s0 = small.tile([D + 1, 512], f32, name="pe_s0", tag="pe_s0")
                    nc.vector.tensor_copy(pe_s0[:, :], pe0[:, :])
                    pe_s1 = small.tile([D + 1, 512], f32, name="pe_s1", tag="pe_s1")
                    nc.vector.tensor_copy(pe_s1[:, :], pe1[:, :])
                    pd_s = small.tile([128, 512], f32, name="pd_s", tag="pd_s")
                    nc.vector.tensor_copy(pd_s[:, :], pd[:, :])
                rec2 = small.tile([1, 1024], f16 if last else f32, name="rec2",
                                  tag="rec2l" if last else "rec2")
                for half, pes in ((0, pe_s0), (1, pe_s1)):
                    with nc.allow_low_precision(reason="0.5/r broadcast"):
                        nc.vector.reciprocal(
                            rec2[:, half * 512:(half + 1) * 512], pes[D:D + 1, :])
                if last:
                    bcp = psS.tile([D, 1024], f32, name="bcp", tag="psS",
                                   padded_shape=[128, 1024])
                    nc.tensor.matmul(bcp[:, 0:512], lhsT=ones16[:, :],
                                     rhs=rec2[:, 0:512], start=True, stop=True)
                    nc.tensor.matmul(bcp[:, 512:1024], lhsT=ones16[:, :],
                                     rhs=rec2[:, 512:1024], start=True, stop=True)
                    bcs = small.tile([D, 1024], f32, name="bcs", tag="bcs")
                    nc.vector.tensor_copy(bcs[:, :], bcp[:, :])
                else:
                    nc.sync.dma_start(out=rscratch[slot:slot + 1, :], in_=rec2[:, :])
                    row = rscratch[slot, :]
                    bc_ap = bass.AP(tensor=row.tensor, offset=row.offset,
                                    ap=[[0, D]] + list(row.ap))
                    bcs = small.tile([D, 1024], f32, name="bcs", tag="bcs")
                    nc.sync.dma_start(out=bcs[:, :], in_=bc_ap)
                for half, pes in ((0, pe_s0), (1, pe_s1)):
                    t1 = small.tile([128, 512], f32, name="t1", tag="t1")
                    nc.vector.tensor_mul(
                        t1[half * D:(half + 1) * D, :], pes[0:D, :],
                        bcs[:, half * 512:(half + 1) * 512])
                    nc.vector.tensor_add(
                        outT[half * D:(half + 1) * D, hp, qsl],
                        t1[half * D:(half + 1) * D, :],
                        pd_s[half * D:(half + 1) * D, :],
                    )
            pending_proj = nq
        for co in range(C // 128):
            proj_group(NQ - 1, co)
    nc.compile()
    return nc


_PROGRAM = None


def _get_program():
    global _PROGRAM
    if _PROGRAM is None:
        _PROGRAM = _build_program()
    return _PROGRAM


def _make_in_maps(x, distance_matrix, W_qkv, W_proj):
    in_maps = []
    for core in range(NCORES):
        b, hg = divmod(core, HG)
        sl = slice(hg * DG, (hg + 1) * DG)
        in_maps.append({
            "xT": np.ascontiguousarray(x[b].T).astype(np.float16),
            "wq": np.ascontiguousarray(W_qkv[:, sl]).astype(np.float16),
            "wk": np.ascontiguousarray(W_qkv[:, C + hg * DG:C + (hg + 1) * DG]).astype(np.float16),
            "wv": np.ascontiguousarray(W_qkv[:, 2 * C + hg * DG:2 * C + (hg + 1) * DG]).astype(np.float16),
            "wp": np.ascontiguousarray(W_proj[sl, :]).astype(np.float16),
            "dmt": np.ascontiguousarray(
                (0.5 * distance_matrix[b, 0].T).astype(np.float16)
            ),
        })
    return in_maps


def kernel(x, distance_matrix, W_qkv, W_proj, b_proj, _results_hook=None):
    from concourse.bass_utils import run_bass_kernel_spmd

    x = np.asarray(x)
    distance_matrix = np.asarray(distance_matrix)
    W_qkv = np.asarray(W_qkv)
    W_proj = np.asarray(W_proj)
    b_proj = np.asarray(b_proj)
    nc = _get_program()
    in_maps = _make_in_maps(x, distance_matrix, W_qkv, W_proj)
    res = run_bass_kernel_spmd(nc, in_maps, list(range(NCORES)))
    if _results_hook is not None:
        _results_hook(res)
    out = np.zeros((B, N, C), dtype=np.float32)
    for core in range(NCORES):
        b = core // HG
        out[b] += res.results[core]["pout"].T
    out += b_proj[None, None, :].astype(np.float32)
    return out



# revision 3
# speedup vs baseline: 1.0882x; 1.0882x over previous
"""Distributed attention kernel for Trainium2 (8 NeuronCores).

Reference computation (B=2, N=2048, C=1024, H=16, D=64, ALPHA=0.5):
    qkv = x @ W_qkv -> q,k,v [B,H,N,D]
    attn = softmax(q @ k^T / sqrt(D))
    attn = 0.5*dm + 0.5*attn
    out  = (attn @ v).reshape(B,N,C) @ W_proj + b_proj

Sharding: 8 cores = 2 batches x 4 head-groups (4 heads each).
Each core computes its head-group's slice end-to-end, including a partial
projection (row-slice of W_proj); host sums the 4 partials per batch.

On-device layout strategy (per core):
  - x arrives transposed [C, N]; q,k are produced transposed [Dg=256, N]
    (head-dim on partitions); scores are computed transposed
    S^T[m, q] = k^T.T @ q^T so exp runs on ScalarE straight out of PSUM.
  - attn@v runs in the *natural* orientation out[q, d] with the exp tile as
    the stationary operand (lhsT = e^T[m, q-tile 128], rhs = v[m, 65]):
    contraction is the full 128 m-rows AND the output uses all 128 q
    partitions, unlike the transposed form whose out has only 65 rows.
    This is the main win over the previous revision (PE cost of attn@v
    halves, and dm@v + normalization get cheaper too).
  - v carries an appended column holding 2.0, so out[q, 64] = 2*r_q (the
    softmax denominator); normalization is a per-partition
    tensor_scalar multiply by 0.5/r_q (vector.reciprocal of the 2r column)
    fused with the dm@v add via scalar_tensor_tensor.
  - dm@v accumulates in the same [q, dg] orientation (lhsT = dm^T tile).
  - The [q, dg] result is transposed back to [dg, q] for the W_proj
    contraction with cheap PE transposes ([128,128] identity matmuls).
  - PSUM budget (8 banks): scores [128,1024] x2 bufs = 4; e@v accumulators
    = 2 banks, each holding two q-subtile groups [128,130] at 256-col
    offsets -- only the first matmul per bank uses start=True (hardware
    zeroes the whole 2KB bank region), every other group accumulates with
    start=False onto pending-zero bytes; 2 "x" banks rotate between
    v-projection, dm@v accumulation, output transposes and W_proj groups.
  - max-subtraction is skipped: scores are ~N(0,1), exp never overflows.
  - all matmul operands are fp16; PSUM accumulation stays fp32.
"""

import numpy as np

B, N, C, H, D = 2, 2048, 1024, 16, 64
NCORES = 8
HG = 4                # head-groups per batch
HPC = H // HG         # heads per core = 4
DG = HPC * D          # 256: head-group width
SCALE = D ** -0.5

KT = C // 128         # 8 contraction tiles for qkv/x
MT = N // 128         # 16 m (key) tiles
NQ = N // 512         # 4 q-chunks
QT = N // 128         # 16 q-tiles


def _build_program():
    import concourse.bass as bass
    import concourse.bacc as bacc
    import concourse.tile as tile
    from concourse import mybir
    from contextlib import ExitStack

    f32 = mybir.dt.float32
    f16 = mybir.dt.float16
    Exp = mybir.ActivationFunctionType.Exp
    Mult = mybir.AluOpType.mult
    Add = mybir.AluOpType.add

    nc = bacc.Bacc()
    xT = nc.declare_dram_parameter("xT", [C, N], f16, isOutput=False)
    wq = nc.declare_dram_parameter("wq", [C, DG], f16, isOutput=False)
    wk = nc.declare_dram_parameter("wk", [C, DG], f16, isOutput=False)
    wv = nc.declare_dram_parameter("wv", [C, DG], f16, isOutput=False)
    wp = nc.declare_dram_parameter("wp", [DG, C], f16, isOutput=False)
    dmt = nc.declare_dram_parameter("dmt", [N, N], f16, isOutput=False)
    ident = nc.declare_dram_parameter("ident", [128, 128], f16, isOutput=False)
    pout = nc.declare_dram_parameter("pout", [C, N], f16, isOutput=True)

    with tile.TileContext(nc) as tc, ExitStack() as ctx:
        big = ctx.enter_context(tc.tile_pool(name="big", bufs=1))
        epool = ctx.enter_context(tc.tile_pool(name="epool", bufs=6))
        small = ctx.enter_context(tc.tile_pool(name="small", bufs=2))
        outp = ctx.enter_context(tc.tile_pool(name="outp", bufs=4))
        # PSUM: psS 2x[128,1024] = 4 banks, psA 2 banks, psX 2 banks.
        psS = ctx.enter_context(tc.tile_pool(name="psS", bufs=2, space="PSUM"))
        psA = ctx.enter_context(tc.tile_pool(name="psA", bufs=1, space="PSUM"))
        psX = ctx.enter_context(tc.tile_pool(name="psX", bufs=1, space="PSUM"))

        xt = big.tile([128, KT, N], f16)
        wq_s = big.tile([128, KT, DG], f16)
        wk_s = big.tile([128, KT, DG], f16)
        wv_s = big.tile([128, KT, DG], f16)
        wp_s = big.tile([128, 2, C], f16)
        dms = big.tile([128, MT, N], f16)
        qt = big.tile([128, 2, N], f16)
        kt = big.tile([128, 2, N], f16)
        vaug = big.tile([128, MT, HPC, D + 1], f16)
        outacc = big.tile([128, QT, DG], f16)
        dmacc = big.tile([128, QT, DG], f16)
        outT = big.tile([128, 2, N], f16)
        ident_s = big.tile([128, 128], f16)
        ones_sb = big.tile([128, MT * HPC], f32)

        nc.vector.memset(ones_sb[:, :], 2.0)
        nc.vector.tensor_copy(vaug[:, :, :, D], ones_sb[:, :])

        # ---- input DMA (wq before wv: q-proj precedes v-proj on the PE) ----
        for ct in range(KT):
            if ct == 0:
                nc.sync.dma_start(out=xt[:, 0, 0:1024], in_=xT[0:128, 0:1024])
                nc.sync.dma_start(out=xt[:, 0, 1024:2048], in_=xT[0:128, 1024:2048])
            else:
                nc.sync.dma_start(out=xt[:, ct, :], in_=xT[ct * 128:(ct + 1) * 128, :])
            nc.sync.dma_start(out=wk_s[:, ct, :], in_=wk[ct * 128:(ct + 1) * 128, :])
        nc.sync.dma_start(out=ident_s[:, :], in_=ident[:, :])
        for ct in range(KT):
            nc.sync.dma_start(out=wq_s[:, ct, :], in_=wq[ct * 128:(ct + 1) * 128, :])
        for ct in range(KT):
            nc.sync.dma_start(out=wv_s[:, ct, :], in_=wv[ct * 128:(ct + 1) * 128, :])
        for jo in range(2):
            nc.sync.dma_start(out=wp_s[:, jo, :], in_=wp[jo * 128:(jo + 1) * 128, :])
        for mt in range(MT):
            nc.sync.dma_start(out=dms[:, mt, :], in_=dmt[mt * 128:(mt + 1) * 128, :])

        # ---- k-proj: first 6 groups accumulate ct-outer across 6 PSUM slots
        # so each arriving xt tile feeds 6 matmuls (PE keeps pace with DMA).
        kgroups = [(jo, nqi) for jo in range(2) for nqi in range(NQ)]
        slotdefs = [(psS, "psS"), (psS, "psS"), (psA, "a0"), (psA, "a1"),
                    (psX, "x0"), (psX, "x1")]
        kps = {}
        for i, g in enumerate(kgroups[:6]):
            pool, tg = slotdefs[i]
            kps[g] = pool.tile([128, 512], f32, name=f"kp{i}", tag=tg)
        for ct in range(KT):
            for jo, nqi in kgroups[:6]:
                nc.tensor.matmul(
                    kps[(jo, nqi)][:, :],
                    lhsT=wk_s[:, ct, jo * 128:(jo + 1) * 128],
                    rhs=xt[:, ct, nqi * 512:(nqi + 1) * 512],
                    start=(ct == 0), stop=(ct == KT - 1),
                )
        for jo, nqi in kgroups[:6]:
            nc.vector.tensor_copy(kt[:, jo, nqi * 512:(nqi + 1) * 512], kps[(jo, nqi)][:, :])
        for jo, nqi in kgroups[6:]:
            ps = psS.tile([128, 512], f32, name="kps2", tag="psS")
            for i in range(KT):
                ct = (nqi + i) % KT
                nc.tensor.matmul(
                    ps[:, :],
                    lhsT=wk_s[:, ct, jo * 128:(jo + 1) * 128],
                    rhs=xt[:, ct, nqi * 512:(nqi + 1) * 512],
                    start=(i == 0), stop=(i == KT - 1),
                )
            nc.vector.tensor_copy(kt[:, jo, nqi * 512:(nqi + 1) * 512], ps[:, :])

        def q_group(jo, nqi):
            ps = psS.tile([128, 512], f32, name="qps", tag="psS")
            for i in range(KT):
                ct = (nqi + i) % KT
                nc.tensor.matmul(
                    ps[:, :],
                    lhsT=wq_s[:, ct, jo * 128:(jo + 1) * 128],
                    rhs=xt[:, ct, nqi * 512:(nqi + 1) * 512],
                    start=(i == 0), stop=(i == KT - 1),
                )
            nc.vector.tensor_scalar_mul(qt[:, jo, nqi * 512:(nqi + 1) * 512], ps[:, :], SCALE)

        for nqi in range(NQ):
            q_group(0, nqi)

        # ---- fills woven into the attention mt-loops ----
        def v_group(mt):
            ps = psX.tile([128, DG], f32, name="vps", tag=f"x{mt % 2}",
                          padded_shape=[128, 512])
            for i in range(KT):
                ct = (mt + i) % KT
                nc.tensor.matmul(
                    ps[:, :],
                    lhsT=xt[:, ct, mt * 128:(mt + 1) * 128],
                    rhs=wv_s[:, ct, :],
                    start=(i == 0), stop=(i == KT - 1),
                )
            nc.vector.tensor_copy(vaug[:, mt, :, 0:D], ps[:, :])

        def make_dm_fill(nqi):
            tiles = [psX.tile([128, 512], f32, name=f"dmps{i}", tag=f"x{i}")
                     for i in range(2)]

            def fill(mt):
                for qs in range(4):
                    qti = nqi * 4 + qs
                    bank = tiles[qs // 2]
                    base = (qs % 2) * 256
                    nc.tensor.matmul(
                        bank[:, base:base + DG],
                        lhsT=dms[:, mt, qti * 128:(qti + 1) * 128],
                        rhs=vaug[:, mt, :, 0:D],
                        start=(mt == 0 and qs % 2 == 0),
                        stop=(mt == MT - 1 and qs % 2 == 1),
                        skip_group_check=True,
                    )

            def finish():
                for i in range(2):
                    q0 = nqi * 4 + 2 * i
                    nc.vector.tensor_copy(dmacc[:, q0:q0 + 2, :], tiles[i][:, :])

            return fill, finish

        def proj_group(nqi, co):
            qsl = slice(nqi * 512, (nqi + 1) * 512)
            ps = psX.tile([128, 512], f32, name="pps", tag=f"x{co % 2}")
            for jo in range(2):
                nc.tensor.matmul(
                    ps[:, :],
                    lhsT=wp_s[:, jo, co * 128:(co + 1) * 128],
                    rhs=outT[:, jo, qsl],
                    start=(jo == 0), stop=(jo == 1),
                )
            so = outp.tile([128, 512], f16, name="so")
            nc.vector.tensor_copy(so[:, :], ps[:, :])
            nc.sync.dma_start(out=pout[co * 128:(co + 1) * 128, qsl], in_=so[:, :])

        def transposes(nqi):
            for qs in range(4):
                qti = nqi * 4 + qs
                for jo in range(2):
                    tr = psX.tile([128, 128], f16, name="tr",
                                  tag=f"x{(qs * 2 + jo) % 2}", padded_shape=[128, 512])
                    nc.tensor.transpose(tr[:, :], outacc[:, qti, jo * 128:(jo + 1) * 128],
                                        ident_s[:, :])
                    nc.vector.tensor_copy(outT[:, jo, qti * 128:(qti + 1) * 128], tr[:, :])

        # ---- attention pass: scores + exp + e@v for one head pair / q-chunk
        def emit_eav(nqi, hp, eav, mt, et):
            for qs in range(4):
                bank = eav[qs // 2]
                base = (qs % 2) * 256
                for h2 in range(2):
                    nc.tensor.matmul(
                        bank[:, base + h2 * 65: base + h2 * 65 + 65],
                        lhsT=et[:, h2 * 512 + qs * 128: h2 * 512 + (qs + 1) * 128],
                        rhs=vaug[:, mt, 2 * hp + h2, :],
                        start=(mt == 0 and qs % 2 == 0 and h2 == 0),
                        stop=(mt == MT - 1 and qs % 2 == 1 and h2 == 1),
                        skip_group_check=True,
                    )

        def attn_pass(nqi, hp, fill):
            qsl = slice(nqi * 512, (nqi + 1) * 512)
            eav = [psA.tile([128, 512], f32, name=f"eav{i}", tag=f"a{i}")
                   for i in range(2)]
            pend = []
            for mt in range(MT):
                if fill is not None:
                    fill(mt)
                msl = slice(mt * 128, (mt + 1) * 128)
                sps = psS.tile([128, 1024], f32, name="sps", tag="psS")
                nc.tensor.matmul(sps[:, 0:512], lhsT=kt[0:D, hp, msl],
                                 rhs=qt[0:D, hp, qsl], start=True, stop=True)
                nc.tensor.matmul(sps[:, 512:1024], lhsT=kt[D:128, hp, msl],
                                 rhs=qt[D:128, hp, qsl], start=True, stop=True)
                et = epool.tile([128, 1024], f16, name="et", tag="et")
                nc.scalar.activation(et[:, :], sps[:, :], Exp)
                pend.append((mt, et))
                if len(pend) > 1:
                    emit_eav(nqi, hp, eav, *pend.pop(0))
            while pend:
                emit_eav(nqi, hp, eav, *pend.pop(0))
            return eav

        def epilogue(nqi, hp, eav, with_dm):
            for qs in range(4):
                qti = nqi * 4 + qs
                bank = eav[qs // 2]
                base = (qs % 2) * 256
                rec = small.tile([128, 2], f32, name="rec", tag="rec")
                with nc.allow_low_precision(reason="0.5/r per-q reciprocal"):
                    for h2 in range(2):
                        nc.vector.reciprocal(rec[:, h2:h2 + 1],
                                             bank[:, base + h2 * 65 + 64: base + h2 * 65 + 65])
                for h2 in range(2):
                    col = base + h2 * 65
                    dst = outacc[:, qti, (2 * hp + h2) * 64:(2 * hp + h2 + 1) * 64]
                    if with_dm:
                        nc.vector.scalar_tensor_tensor(
                            dst, bank[:, col:col + 64], rec[:, h2:h2 + 1],
                            dmacc[:, qti, (2 * hp + h2) * 64:(2 * hp + h2 + 1) * 64],
                            op0=Mult, op1=Add)
                    else:
                        nc.vector.tensor_scalar_mul(dst, bank[:, col:col + 64],
                                                    rec[:, h2:h2 + 1])

        # ---- main schedule ----
        # nq0/hp0 weaves the v-projection; nq0/hp1 weaves dm@v for nq0 (its
        # hp0 epilogue lacks the dm term -- fixed up after hp1).  For nq>=1:
        # hp0 weaves dm@v(nq), hp1 weaves the W_proj groups of nq-1.
        eav = attn_pass(0, 0, v_group)
        epilogue(0, 0, eav, with_dm=False)
        q_group(1, 0)
        dmfill, dmfin = make_dm_fill(0)
        eav = attn_pass(0, 1, dmfill)
        dmfin()
        epilogue(0, 1, eav, with_dm=True)
        for qs in range(4):
            nc.vector.tensor_add(outacc[:, qs, 0:128], outacc[:, qs, 0:128],
                                 dmacc[:, qs, 0:128])
        q_group(1, 1)
        transposes(0)
        for nqi in range(1, NQ):
            dmfill, dmfin = make_dm_fill(nqi)
            eav = attn_pass(nqi, 0, dmfill)
            dmfin()
            epilogue(nqi, 0, eav, with_dm=True)
            if nqi + 1 < NQ:
                q_group(1, nqi + 1)

            def pfill(mt, _p=nqi - 1):
                if mt % 2 == 0:
                    proj_group(_p, mt // 2)

            eav = attn_pass(nqi, 1, pfill)
            epilogue(nqi, 1, eav, with_dm=True)
            transposes(nqi)
        for co in range(8):
            proj_group(NQ - 1, co)
    nc.compile()
    return nc


_PROGRAM = None


def _get_program():
    global _PROGRAM
    if _PROGRAM is None:
        _PROGRAM = _build_program()
    return _PROGRAM


def _make_in_maps(x, distance_matrix, W_qkv, W_proj):
    ident = np.eye(128, dtype=np.float16)
    in_maps = []
    for core in range(NCORES):
        b, hg = divmod(core, HG)
        sl = slice(hg * DG, (hg + 1) * DG)
        in_maps.append({
            "xT": np.ascontiguousarray(x[b].T).astype(np.float16),
            "wq": np.ascontiguousarray(W_qkv[:, sl]).astype(np.float16),
            "wk": np.ascontiguousarray(W_qkv[:, C + hg * DG:C + (hg + 1) * DG]).astype(np.float16),
            "wv": np.ascontiguousarray(W_qkv[:, 2 * C + hg * DG:2 * C + (hg + 1) * DG]).astype(np.float16),
            "wp": np.ascontiguousarray(W_proj[sl, :]).astype(np.float16),
            "dmt": np.ascontiguousarray(
                (0.5 * distance_matrix[b, 0].T).astype(np.float16)
            ),
            "ident": ident,
        })
    return in_maps


def kernel(x, distance_matrix, W_qkv, W_proj, b_proj, _results_hook=None):
    from concourse.bass_utils import run_bass_kernel_spmd

    x = np.asarray(x)
    distance_matrix = np.asarray(distance_matrix)
    W_qkv = np.asarray(W_qkv)
    W_proj = np.asarray(W_proj)
    b_proj = np.asarray(b_proj)
    nc = _get_program()
    in_maps = _make_in_maps(x, distance_matrix, W_qkv, W_proj)
    res = run_bass_kernel_spmd(nc, in_maps, list(range(NCORES)))
    if _results_hook is not None:
        _results_hook(res)
    out = np.zeros((B, N, C), dtype=np.float32)
    for core in range(NCORES):
        b = core // HG
        out[b] += res.results[core]["pout"].T
    out += b_proj[None, None, :].astype(np.float32)
    return out


# revision 7
# speedup vs baseline: 1.0953x; 1.0066x over previous
"""Distributed attention kernel for Trainium2 (8 NeuronCores).

Reference computation (B=2, N=2048, C=1024, H=16, D=64, ALPHA=0.5):
    qkv = x @ W_qkv -> q,k,v [B,H,N,D]
    attn = softmax(q @ k^T / sqrt(D))
    attn = 0.5*dm + 0.5*attn
    out  = (attn @ v).reshape(B,N,C) @ W_proj + b_proj

Sharding: 8 cores = 2 batches x 4 head-groups (4 heads each).
Each core computes its head-group's slice end-to-end, including a partial
projection (row-slice of W_proj); host sums the 4 partials per batch.

On-device layout strategy (per core):
  - x arrives transposed [C, N]; q,k are produced transposed [Dg=256, N]
    (head-dim on partitions); scores are computed transposed
    S^T[m, q] = k^T.T @ q^T so exp runs on ScalarE straight out of PSUM.
  - attn@v runs in the *natural* orientation out[q, d] with the exp tile as
    the stationary operand (lhsT = e^T[m, q-tile 128], rhs = v[m, 65]):
    contraction is the full 128 m-rows AND the output uses all 128 q
    partitions (the transposed form only fills 65 of 128 output rows).
  - v carries an appended column holding 2.0, so out[q, 64] = 2*r_q (the
    softmax denominator); normalization is a per-partition multiply by
    0.5/r_q (vector.reciprocal of the 2r column) fused with the dm@v add
    via scalar_tensor_tensor.
  - dm@v runs in fp8(e4m3) DoubleRow perf mode (0.5 cycles/row): dm is
    pre-scaled by 0.5*256, transposed and pair-packed on the host
    ([64 partitions, 2 rows] layout, duplicated across both partition
    halves for v); v is quantized to fp8 on device and pair-packed with a
    small SBUF->SBUF DMA.  The 1/256 rescale rides the PSUM->SBUF copy.
    (fp8 on the softmax path fails the 2e-2 gate -- measured 4.4e-2 -- but
    the dm path alone measures ~1e-2.)
  - The [q, dg] result is transposed back to [dg, q] for the W_proj
    contraction with cheap PE transposes ([128,128] identity matmuls).
  - PSUM budget (8 banks): scores [128,1024] x2 bufs = 4; e@v accumulators
    = 2 banks, each holding two q-subtile groups [128,130] at 256-col
    offsets -- only the first matmul per bank uses start=True (hardware
    zeroes the whole 2KB bank region), every other group accumulates with
    start=False onto pending-zero bytes; 2 "x" banks rotate between
    v-projection, dm@v accumulation, output transposes and W_proj groups.
  - DMA is batched: weights/dm arrive pre-packed host-side so each tensor
    is one (or a few) large descriptors-contiguous transfers; the output
    is staged per q-chunk and stored with one DMA per chunk.  (Each DMA
    instruction costs ~625ns on the shared HWDGE issue path, so the
    previous ~50 small input DMAs serialized the prologue.)
  - max-subtraction is skipped: scores are ~N(0,1), exp never overflows.
  - softmax-path matmuls are fp16; PSUM accumulation stays fp32.
"""

import numpy as np

B, N, C, H, D = 2, 2048, 1024, 16, 64
NCORES = 8
HG = 4                # head-groups per batch
HPC = H // HG         # heads per core = 4
DG = HPC * D          # 256: head-group width
SCALE = D ** -0.5
DM_SCALE = 256.0

KT = C // 128         # 8 contraction tiles for qkv/x
MT = N // 128         # 16 m (key) tiles
NQ = N // 512         # 4 q-chunks
QT = N // 128         # 16 q-tiles


def _build_program():
    import concourse.bass as bass
    import concourse.bacc as bacc
    import concourse.tile as tile
    from concourse import mybir
    from contextlib import ExitStack

    f32 = mybir.dt.float32
    f16 = mybir.dt.float16
    f8 = mybir.dt.float8e4
    Exp = mybir.ActivationFunctionType.Exp
    Mult = mybir.AluOpType.mult
    Add = mybir.AluOpType.add
    DR = mybir.MatmulPerfMode.DoubleRow

    nc = bacc.Bacc()
    xT = nc.declare_dram_parameter("xT", [C, N], f16, isOutput=False)
    wq = nc.declare_dram_parameter("wq", [128, KT * DG], f16, isOutput=False)
    wk = nc.declare_dram_parameter("wk", [128, KT * DG], f16, isOutput=False)
    wv = nc.declare_dram_parameter("wv", [128, KT * DG], f16, isOutput=False)
    wp = nc.declare_dram_parameter("wp", [128, 2 * C], f16, isOutput=False)
    dm8 = nc.declare_dram_parameter("dm8", [128, (MT // 2) * 2 * N], f8, isOutput=False)
    ident = nc.declare_dram_parameter("ident", [128, 128], f16, isOutput=False)
    pout = nc.declare_dram_parameter("pout", [C, N], f16, isOutput=True)

    with tile.TileContext(nc) as tc, ExitStack() as ctx:
        big = ctx.enter_context(tc.tile_pool(name="big", bufs=1))
        epool = ctx.enter_context(tc.tile_pool(name="epool", bufs=6))
        small = ctx.enter_context(tc.tile_pool(name="small", bufs=2))
        outp = ctx.enter_context(tc.tile_pool(name="outp", bufs=2))
        # PSUM: psS 2x[128,1024] = 4 banks, psA 2 banks, psX 2 banks.
        psS = ctx.enter_context(tc.tile_pool(name="psS", bufs=2, space="PSUM"))
        psA = ctx.enter_context(tc.tile_pool(name="psA", bufs=1, space="PSUM"))
        psX = ctx.enter_context(tc.tile_pool(name="psX", bufs=1, space="PSUM"))

        xt = big.tile([128, KT, N], f16)
        wq_s = big.tile([128, KT, DG], f16)
        wk_s = big.tile([128, KT, DG], f16)
        wv_s = big.tile([128, KT, DG], f16)
        wp_s = big.tile([128, 2, C], f16)
        dms8 = big.tile([128, MT // 2, 2, N], f8)
        qt = big.tile([128, 2, N], f16)
        kt = big.tile([128, 2, N], f16)
        vaug = big.tile([128, MT, HPC, D + 1], f16)
        v8t = big.tile([128, MT, DG], f8)
        v8 = big.tile([128, MT, 2, DG], f8)
        outacc = big.tile([128, QT, DG], f16)
        dmacc = big.tile([128, QT, DG], f16)
        outT = big.tile([128, 2, N], f16)
        ident_s = big.tile([128, 128], f16)
        ones_sb = big.tile([128, MT * HPC], f32)

        nc.vector.memset(ones_sb[:, :], 2.0)
        nc.vector.tensor_copy(vaug[:, :, :, D], ones_sb[:, :])

        def dram_ap(t, offset, dims):
            base = t[:, :]
            return bass.AP(tensor=base.tensor, offset=base.offset + offset, ap=dims)

        # ---- input DMA, batched.  x in 4 chunks of 2 ct; weights one DMA
        # each (host-packed rows); dm8 in 4 chunks of 2 mt-pairs.
        def x_chunk(c):
            nc.sync.dma_start(
                out=xt[:, 2 * c:2 * c + 2, :],
                in_=dram_ap(xT, 2 * c * 128 * N, [[N, 128], [128 * N, 2], [1, N]]))

        x_chunk(0)
        nc.sync.dma_start(out=wk_s[:, :, :], in_=wk[:, :])
        nc.sync.dma_start(out=wq_s[:, :, :], in_=wq[:, :])
        for c in range(1, 4):
            x_chunk(c)
        nc.sync.dma_start(out=wv_s[:, :, :], in_=wv[:, :])
        nc.sync.dma_start(out=wp_s[:, :, :], in_=wp[:, :])
        nc.sync.dma_start(out=ident_s[:, :], in_=ident[:, :])
        for c in range(4):
            nc.sync.dma_start(out=dms8[:, 2 * c:2 * c + 2, :, :],
                              in_=dm8[:, 2 * c * 2 * N:(2 * c + 2) * 2 * N])

        # ---- k-proj: first 6 groups accumulate ct-outer across 6 PSUM slots
        # so each arriving xt chunk feeds many matmuls.
        kgroups = [(jo, nqi) for jo in range(2) for nqi in range(NQ)]
        slotdefs = [(psS, "psS"), (psS, "psS"), (psA, "a0"), (psA, "a1"),
                    (psX, "x0"), (psX, "x1")]
        kps = {}
        for i, g in enumerate(kgroups[:6]):
            pool, tg = slotdefs[i]
            kps[g] = pool.tile([128, 512], f32, name=f"kp{i}", tag=tg)
        for ct in range(KT):
            for jo, nqi in kgroups[:6]:
                nc.tensor.matmul(
                    kps[(jo, nqi)][:, :],
                    lhsT=wk_s[:, ct, jo * 128:(jo + 1) * 128],
                    rhs=xt[:, ct, nqi * 512:(nqi + 1) * 512],
                    start=(ct == 0), stop=(ct == KT - 1),
                )
        for jo, nqi in kgroups[:6]:
            nc.vector.tensor_copy(kt[:, jo, nqi * 512:(nqi + 1) * 512], kps[(jo, nqi)][:, :])

        def k_group(jo, nqi):
            ps = psS.tile([128, 512], f32, name="kps2", tag="psS")
            for i in range(KT):
                ct = (nqi + i) % KT
                nc.tensor.matmul(
                    ps[:, :],
                    lhsT=wk_s[:, ct, jo * 128:(jo + 1) * 128],
                    rhs=xt[:, ct, nqi * 512:(nqi + 1) * 512],
                    start=(i == 0), stop=(i == KT - 1),
                )
            nc.vector.tensor_copy(kt[:, jo, nqi * 512:(nqi + 1) * 512], ps[:, :])

        def q_group(jo, nqi):
            ps = psS.tile([128, 512], f32, name="qps", tag="psS")
            for i in range(KT):
                ct = (nqi + i) % KT
                nc.tensor.matmul(
                    ps[:, :],
                    lhsT=wq_s[:, ct, jo * 128:(jo + 1) * 128],
                    rhs=xt[:, ct, nqi * 512:(nqi + 1) * 512],
                    start=(i == 0), stop=(i == KT - 1),
                )
            nc.vector.tensor_scalar_mul(qt[:, jo, nqi * 512:(nqi + 1) * 512], ps[:, :], SCALE)

        q_group(0, 0)
        # remaining prologue groups, woven into the first attention pass
        pro_fill = [lambda nqi=nqi: q_group(0, nqi) for nqi in (1, 2, 3)]
        pro_fill += [lambda g=g: k_group(*g) for g in kgroups[6:]]

        # ---- fills woven into the attention mt-loops ----
        def v_group(mt):
            ps = psX.tile([128, DG], f32, name="vps", tag=f"x{mt % 2}",
                          padded_shape=[128, 512])
            for i in range(KT):
                ct = (mt + i) % KT
                nc.tensor.matmul(
                    ps[:, :],
                    lhsT=xt[:, ct, mt * 128:(mt + 1) * 128],
                    rhs=wv_s[:, ct, :],
                    start=(i == 0), stop=(i == KT - 1),
                )
            nc.vector.tensor_copy(vaug[:, mt, :, 0:D], ps[:, :])
            nc.vector.tensor_copy(v8t[:, mt, :], ps[:, :])

        def v8_remap(half):
            # pair-pack v8t [128m, mt, d] -> v8 [64, mt, 2, d] duplicated on
            # both partition halves (DoubleRow wants lhsT/rhs at the same
            # base partition).  half selects mt 0:8 or 8:16.
            ms = slice(half * 8, (half + 1) * 8)
            for dup in range(2):
                for i in range(2):
                    nc.sync.dma_start(
                        out=v8[dup * 64:(dup + 1) * 64, ms, i, :],
                        in_=v8t[i * 64:(i + 1) * 64, ms, :])

        def make_dm_fill(nqi, shift=0):
            tiles = [psX.tile([128, 512], f32, name=f"dmps{i}", tag=f"x{i}")
                     for i in range(2)]

            def step(mm):
                pb = (mm % 2) * 64
                for qs in range(4):
                    qti = nqi * 4 + qs
                    bank = tiles[qs // 2]
                    base = (qs % 2) * 256
                    nc.tensor.matmul(
                        bank[:, base:base + DG],
                        lhsT=dms8[pb:pb + 64, mm // 2, :, qti * 128:(qti + 1) * 128],
                        rhs=v8[pb:pb + 64, mm, :, :],
                        start=(mm == 0 and qs % 2 == 0),
                        stop=(mm == MT - 1 and qs % 2 == 1),
                        perf_mode=DR,
                        skip_group_check=True,
                    )

            def fill(mt):
                if mt >= shift:
                    step(mt - shift)

            def finish():
                for mm in range(MT - shift, MT):
                    step(mm)
                for i in range(2):
                    q0 = nqi * 4 + 2 * i
                    nc.vector.tensor_scalar_mul(dmacc[:, q0:q0 + 2, :], tiles[i][:, :],
                                                1.0 / DM_SCALE)

            return fill, finish

        def new_so(nqi):
            return outp.tile([128, 8, 512], f16, name="so", tag=f"so{nqi % 2}")

        def proj_group(nqi, co, so, tags=("x0", "x1")):
            qsl = slice(nqi * 512, (nqi + 1) * 512)
            tg = tags[co % len(tags)]
            pool = psA if tg.startswith("a") else psX
            ps = pool.tile([128, 512], f32, name="pps", tag=tg)
            for jo in range(2):
                nc.tensor.matmul(
                    ps[:, :],
                    lhsT=wp_s[:, jo, co * 128:(co + 1) * 128],
                    rhs=outT[:, jo, qsl],
                    start=(jo == 0), stop=(jo == 1),
                )
            nc.vector.tensor_copy(so[:, co, :], ps[:, :])
            if co == 7:
                nc.sync.dma_start(
                    out=dram_ap(pout, nqi * 512, [[N, 128], [128 * N, 8], [1, 512]]),
                    in_=so[:, :, :])

        def transposes(nqi):
            for qs in range(4):
                qti = nqi * 4 + qs
                for jo in range(2):
                    tr = psX.tile([128, 128], f16, name="tr",
                                  tag=f"x{(qs * 2 + jo) % 2}", padded_shape=[128, 512])
                    nc.tensor.transpose(tr[:, :], outacc[:, qti, jo * 128:(jo + 1) * 128],
                                        ident_s[:, :])
                    nc.vector.tensor_copy(outT[:, jo, qti * 128:(qti + 1) * 128], tr[:, :])

        # ---- attention pass: scores + exp + e@v for one head pair / q-chunk
        def emit_eav(nqi, hp, eav, mt, et):
            for qs in range(4):
                bank = eav[qs // 2]
                base = (qs % 2) * 256
                for h2 in range(2):
                    nc.tensor.matmul(
                        bank[:, base + h2 * 65: base + h2 * 65 + 65],
                        lhsT=et[:, h2 * 512 + qs * 128: h2 * 512 + (qs + 1) * 128],
                        rhs=vaug[:, mt, 2 * hp + h2, :],
                        start=(mt == 0 and qs % 2 == 0 and h2 == 0),
                        stop=(mt == MT - 1 and qs % 2 == 1 and h2 == 1),
                        skip_group_check=True,
                    )

        def attn_pass(nqi, hp, fill):
            qsl = slice(nqi * 512, (nqi + 1) * 512)
            eav = [psA.tile([128, 512], f32, name=f"eav{i}", tag=f"a{i}")
                   for i in range(2)]
            pend = []
            for mt in range(MT):
                if fill is not None:
                    fill(mt)
                msl = slice(mt * 128, (mt + 1) * 128)
                sps = psS.tile([128, 1024], f32, name="sps", tag="psS")
                nc.tensor.matmul(sps[:, 0:512], lhsT=kt[0:D, hp, msl],
                                 rhs=qt[0:D, hp, qsl], start=True, stop=True)
                nc.tensor.matmul(sps[:, 512:1024], lhsT=kt[D:128, hp, msl],
                                 rhs=qt[D:128, hp, qsl], start=True, stop=True)
                et = epool.tile([128, 1024], f16, name="et", tag="et")
                nc.scalar.activation(et[:, :], sps[:, :], Exp)
                pend.append((mt, et))
                if len(pend) > 1:
                    emit_eav(nqi, hp, eav, *pend.pop(0))
            while pend:
                emit_eav(nqi, hp, eav, *pend.pop(0))
            return eav

        def epilogue(nqi, hp, eav, with_dm):
            for qs in range(4):
                qti = nqi * 4 + qs
                bank = eav[qs // 2]
                base = (qs % 2) * 256
                rec = small.tile([128, 2], f32, name="rec", tag="rec")
                with nc.allow_low_precision(reason="0.5/r per-q reciprocal"):
                    for h2 in range(2):
                        nc.vector.reciprocal(rec[:, h2:h2 + 1],
                                             bank[:, base + h2 * 65 + 64: base + h2 * 65 + 65])
                for h2 in range(2):
                    col = base + h2 * 65
                    dst = outacc[:, qti, (2 * hp + h2) * 64:(2 * hp + h2 + 1) * 64]
                    if with_dm:
                        nc.vector.scalar_tensor_tensor(
                            dst, bank[:, col:col + 64], rec[:, h2:h2 + 1],
                            dmacc[:, qti, (2 * hp + h2) * 64:(2 * hp + h2 + 1) * 64],
                            op0=Mult, op1=Add)
                    else:
                        nc.vector.tensor_scalar_mul(dst, bank[:, col:col + 64],
                                                    rec[:, h2:h2 + 1])

        # ---- main schedule ----
        # nq0/hp0 weaves the v-projection + leftover prologue groups; nq0/hp1
        # weaves dm@v(nq0) (its hp0 epilogue lacks the dm term -- fixed up
        # after hp1).  For nq>=1: hp0 weaves dm@v(nq), hp1 weaves the W_proj
        # groups of nq-1.
        def fill00(mt):
            v_group(mt)
            if mt == 9:
                v8_remap(0)
            if mt % 3 == 1 and pro_fill:
                pro_fill.pop(0)()

        eav = attn_pass(0, 0, fill00)
        while pro_fill:
            pro_fill.pop(0)()
        v8_remap(1)
        epilogue(0, 0, eav, with_dm=False)
        q_group(1, 0)
        dmfill, dmfin = make_dm_fill(0, shift=2)
        eav = attn_pass(0, 1, dmfill)
        dmfin()
        epilogue(0, 1, eav, with_dm=True)
        for qs in range(4):
            nc.vector.tensor_add(outacc[:, qs, 0:128], outacc[:, qs, 0:128],
                                 dmacc[:, qs, 0:128])
        q_group(1, 1)
        transposes(0)
        for nqi in range(1, NQ):
            dmfill, dmfin = make_dm_fill(nqi)
            eav = attn_pass(nqi, 0, dmfill)
            dmfin()
            epilogue(nqi, 0, eav, with_dm=True)
            if nqi + 1 < NQ:
                q_group(1, nqi + 1)
            so = new_so(nqi - 1)

            def pfill(mt, _p=nqi - 1, _so=so):
                if mt % 2 == 0:
                    proj_group(_p, mt // 2, _so)

            eav = attn_pass(nqi, 1, pfill)
            epilogue(nqi, 1, eav, with_dm=True)
            transposes(nqi)
        so = new_so(NQ - 1)
        for co in range(8):
            proj_group(NQ - 1, co, so, tags=("a0", "a1", "x0", "x1"))
    nc.compile()
    return nc


_PROGRAM = None


def _get_program():
    global _PROGRAM
    if _PROGRAM is None:
        _PROGRAM = _build_program()
    return _PROGRAM


def _pack_rows(w, kt):
    # [kt*128, F] -> [128, kt*F]: partition p holds rows p, 128+p, ...
    F = w.shape[1]
    return np.ascontiguousarray(
        w.reshape(kt, 128, F).transpose(1, 0, 2).reshape(128, kt * F))


def _make_in_maps(x, distance_matrix, W_qkv, W_proj):
    import ml_dtypes

    ident = np.eye(128, dtype=np.float16)
    in_maps = []
    for core in range(NCORES):
        b, hg = divmod(core, HG)
        sl = slice(hg * DG, (hg + 1) * DG)
        dmT = (0.5 * DM_SCALE) * distance_matrix[b, 0].T.astype(np.float32)
        # [m, q] -> [128, mtp, i, q]: partition pb*64+p64 holds row
        # m = (2*mtp+pb)*128 + i*64 + p64  (DoubleRow pair packing)
        dmp = dmT.reshape(MT // 2, 2, 2, 64, N).transpose(3, 1, 0, 2, 4)
        dmp = np.ascontiguousarray(dmp.transpose(1, 0, 2, 3, 4).reshape(128, -1))
        in_maps.append({
            "xT": np.ascontiguousarray(x[b].T).astype(np.float16),
            "wq": _pack_rows(W_qkv[:, sl].astype(np.float16), KT),
            "wk": _pack_rows(W_qkv[:, C + hg * DG:C + (hg + 1) * DG].astype(np.float16), KT),
            "wv": _pack_rows(W_qkv[:, 2 * C + hg * DG:2 * C + (hg + 1) * DG].astype(np.float16), KT),
            "wp": _pack_rows(W_proj[sl, :].astype(np.float16), 2),
            "dm8": dmp.astype(ml_dtypes.float8_e4m3),
            "ident": ident,
        })
    return in_maps


def kernel(x, distance_matrix, W_qkv, W_proj, b_proj, _results_hook=None):
    from concourse.bass_utils import run_bass_kernel_spmd

    x = np.asarray(x)
    distance_matrix = np.asarray(distance_matrix)
    W_qkv = np.asarray(W_qkv)
    W_proj = np.asarray(W_proj)
    b_proj = np.asarray(b_proj)
    nc = _get_program()
    in_maps = _make_in_maps(x, distance_matrix, W_qkv, W_proj)
    res = run_bass_kernel_spmd(nc, in_maps, list(range(NCORES)))
    if _results_hook is not None:
        _results_hook(res)
    out = np.zeros((B, N, C), dtype=np.float32)
    for core in range(NCORES):
        b = core // HG
        out[b] += res.results[core]["pout"].T
    out += b_proj[None, None, :].astype(np.float32)
    return out
